# revision 4
# baseline (speedup 1.0000x reference)
"""Correlation (FlowNet-style, max_displacement=4) on 8 TRN2 NeuronCores.

Full inputs x1, x2: [B=8, C=64, H=192, W=192] fp32. Output: [8, 81, 192, 192] fp32.
out[b, di*9+dj, h, w] = mean_c x1[b,c,h,w] * x2pad[b,c,h+di,w+dj]   (di,dj in [0,9))

Strategy: batch-parallel (1 batch per core). Per core the correlation is a banded
Gram computed on the TensorEngine with narrow tiles: for each 8x4 (h,w) output
tile (M=32 pixels), one bf16 matmul with lhsT = x1 tile [K=64, M=32] and
rhs = padded x2 window [64, 16x12=192] yields all 81 displacement dot products
of each tile pixel inside a skewed band of the 32x192 PSUM block. Narrow tiles
(TW=4) shrink both the PSUM volume that must be evicted by DVE/ACT
(192 elem/pixel vs 384 for 8x16 tiles) and the band parallelogram written to
HBM (1.33x the useful bytes vs 2.67x).

The 128-wide PE array is filled via tiling: the h axis is split into two halves
(channels on partitions 0-63 / 64-127 -> row groups), and four w-tiles are
column-tiled into the four 32-partition PSUM column groups, so 8 narrow matmuls
run concurrently on disjoint PE tiles. A [128, 1024] PSUM tile (2 banks) holds
16 matmuls (4 w-tiles x 4 col groups); one DVE/ACT copy evicts it (fp32->bf16)
into a strip-group ybuf. Only the per-th band rectangles (108 of 192 columns)
are DMA'd out, batched over 4 strips (41.5 KB per DMA). The band is deskewed on
the host with a zero-copy strided view; x1 is pre-scaled by 1/64 (exact).
"""

import sys
import types

import numpy as np
import ml_dtypes

import concourse.bacc as bacc
from concourse import mybir
from concourse.tile import TileContext
from concourse.bass_utils import run_bass_kernel_spmd

B, C, H, W = 8, 64, 192, 192
MAXD = 4
D = 2 * MAXD + 1              # 9
HP, WP = H + 2 * MAXD, W + 2 * MAXD  # 200, 200

TH, TW = 8, 4                 # output tile (h, w) -> M = 32
NH, NW = TH + 2 * MAXD, TW + 2 * MAXD  # 16 x 12 window -> N = 192
N = NH * NW                   # 192 matmul free size
M = TH * TW                   # 32 pixels per matmul
NSP = (H // 2) // TH          # 12 strips per partition-half
NT = W // TW                  # 48 w-tiles
HHALF = H // 2                # 96 rows per half
SLAB = HHALF + 2 * MAXD       # 104 padded x2 rows per half
NG = 3                        # strip groups (4 strips each) per half
KL = D * NW                   # 108 band columns per th-group

# Input h-chunking: strip ranges per chunk and the x2 slab rows they need.
X1_CHUNKS = [(0, 3), (3, 9), (9, 12)]
X2_CHUNKS = [(0, 40), (24, 88), (72, 104)]

BF16 = ml_dtypes.bfloat16


def _install_axon_trace_shim():
    """The image's antenv package lacks axon_hooks; run_bass_kernel_spmd
    crashes on import when trace=True. Provide the hook from the boot module
    so tracing works instead of raising."""
    if "antenv.axon_hooks" in sys.modules:
        return
    try:
        import trn_agent_boot.trn_boot as tb

        hook = tb._ntff_profile_via_ctypes("/opt/axon/libaxon_pjrt.so")
    except Exception:
        hook = None
    mod = types.ModuleType("antenv.axon_hooks")
    mod.get_axon_ntff_profile_hook = lambda: hook
    mod.set_axon_ntff_profile_hook = lambda h: None
    sys.modules["antenv.axon_hooks"] = mod


def build_nc():
    nc = bacc.Bacc("TRN2", target_bir_lowering=False, debug=False)
    # x1 arrives pre-tiled: [128 = half*64+c, strip, wtile, 32 pixels] — walrus
    # requires the matmul weights AP to have a single free dimension.
    x1s = nc.dram_tensor("x1s", [128, NSP, NT, M], mybir.dt.bfloat16, kind="ExternalInput")
    x2s = nc.dram_tensor("x2s", [128, SLAB, WP], mybir.dt.bfloat16, kind="ExternalInput")
    y = nc.dram_tensor("y", [NG, 2, TH, 4, TW, KL, 4, 12], mybir.dt.bfloat16,
                       kind="ExternalOutput")

    with TileContext(nc) as tc:
        with (
            tc.tile_pool(name="imgs", bufs=1) as imgs,
            tc.tile_pool(name="outs", bufs=2) as outs,
            tc.tile_pool(name="psum", bufs=4, space="PSUM") as psum,
        ):
            # Chunked input tiles (separate tiles -> precise chunk->matmul deps).
            x1c, x2c = [], []
            for ci in range(3):
                s0, s1 = X1_CHUNKS[ci]
                r0, r1 = X2_CHUNKS[ci]
                x2t = imgs.tile([128, r1 - r0, WP], mybir.dt.bfloat16, tag=f"x2c{ci}")
                nc.sync.dma_start(out=x2t[:], in_=x2s[:, r0:r1, :])
                x1t = imgs.tile([128, s1 - s0, NT, M], mybir.dt.bfloat16, tag=f"x1c{ci}")
                nc.sync.dma_start(out=x1t[:], in_=x1s[:, s0:s1])
                x2c.append(x2t)
                x1c.append(x1t)

            copy_k = 0
            for g in range(NG):
                # ybuf: [128, n, strip-in-group, tau] — tau = local w-tile index;
                # global w-tile t = tau*4 + colgroup (colgroup is folded into the
                # partition index, the host deskew accounts for it).
                ybufs = [outs.tile([128, N, 4, 12], mybir.dt.bfloat16,
                                   name=f"ybuf{half}_{g}", tag=f"ybuf{half}")
                         for half in range(2)]
                for si in range(4):
                    sp = g * 4 + si
                    ci = next(i for i, (s0, s1) in enumerate(X1_CHUNKS) if s0 <= sp < s1)
                    hl = sp * TH - X2_CHUNKS[ci][0]   # row offset within x2 chunk
                    spl = sp - X1_CHUNKS[ci][0]       # strip offset within x1 chunk
                    for jj in range(3):
                        pts = [psum.tile([128, 1024], mybir.dt.float32,
                                         name=f"pt{half}_{sp}_{jj}", tag="pt")
                               for half in range(2)]
                        for aa in range(2):           # PSUM bank within tile
                            for ss in range(2):       # slot within bank
                                tau = 4 * jj + 2 * aa + ss
                                # Interleave halves and col groups: 8 consecutive
                                # matmuls hit 8 disjoint PE (row, col) tile
                                # positions and execute concurrently.
                                for half in range(2):
                                    p0 = 64 * half
                                    off = aa * 512 + ss * N
                                    for c in range(4):
                                        t = tau * 4 + c
                                        nc.tensor.matmul(
                                            pts[half][32 * c:32 * c + 32, off:off + N],
                                            lhsT=x1c[ci][p0:p0 + 64, spl, t, :],
                                            rhs=x2c[ci][p0:p0 + 64, hl:hl + NH,
                                                        t * TW:t * TW + NW],
                                            start=True, stop=True,
                                            tile_position=(p0, 32 * c),
                                        )
                        # Evict both banks with one op; split DVE / ACT 4:5
                        # (DVE is slower per element on PSUM reads).
                        for half in range(2):
                            src = (pts[half][:]
                                   .rearrange("p (a b) -> p a b", a=2)[:, :, 0:2 * N]
                                   .rearrange("p a (s k) -> p a s k", s=2))
                            dst = (ybufs[half][:, :, si, 4 * jj:4 * jj + 4]
                                   .rearrange("p k (a s) -> p a s k", a=2))
                            if copy_k % 9 in (0, 2, 4, 6):
                                nc.vector.tensor_copy(dst, src)
                            else:
                                nc.scalar.copy(dst, src)
                            copy_k += 1
                # Band parallelogram out: per th-group, columns [12*th, 12*th+108)
                # of partitions [32c+4*th, +4) hold all (di, dj) results for those
                # rows — one contiguous 5184B run per partition, 4 strips batched.
                for half in range(2):
                    for th in range(TH):
                        for c in range(4):
                            pb = 32 * c + 4 * th
                            nc.sync.dma_start(
                                out=y[g, half, th, c],
                                in_=ybufs[half][pb:pb + 4, NW * th:NW * th + KL, :, :],
                            )

    nc.compile()
    return nc


_NC_CACHE = None


def _get_nc():
    global _NC_CACHE
    if _NC_CACHE is None:
        _NC_CACHE = build_nc()
    return _NC_CACHE


def _prep_inputs(x1, x2):
    """Host-side shard prep: scale, pad, split h into partition halves, bf16."""
    in_maps = []
    x1 = np.asarray(x1, dtype=np.float32)
    x2 = np.asarray(x2, dtype=np.float32)
    x1h = (x1 * (1.0 / C)).astype(BF16)
    x2h = x2.astype(BF16)
    for b in range(B):
        # x1: [64, 192, 192] -> pre-tiled [128 = half*64+c, sp, t, th*4+tw]
        a = x1h[b].reshape(C, 2, NSP, TH, NT, TW)
        a = a.transpose(1, 0, 2, 4, 3, 5).reshape(128, NSP, NT, M)
        # x2: pad to [64, 200, 200], two overlapping 104-row slabs
        p = np.zeros((C, HP, WP), dtype=BF16)
        p[:, MAXD:MAXD + H, MAXD:MAXD + W] = x2h[b]
        s = np.stack([p[:, 0:SLAB, :], p[:, HHALF:HHALF + SLAB, :]], axis=0)
        s = s.reshape(2 * C, SLAB, WP)
        in_maps.append({"x1s": np.ascontiguousarray(a), "x2s": np.ascontiguousarray(s)})
    return in_maps


def _deskew(yb):
    """yb: [3, 2, 8, 4, 4, 108, 4, 12] fp32 (one batch) -> [81, 192, 192]."""
    st = yb.strides  # (g, half, th, c, tw, kl, s, tau)
    v = np.lib.stride_tricks.as_strided(
        yb,
        shape=(D, D, 2, NG, 4, TH, 12, 4, TW),
        strides=(st[5] * NW, st[5], st[1], st[0], st[6], st[2],
                 st[7], st[3], st[4] + st[5]),
    )
    return np.ascontiguousarray(v).reshape(D * D, H, W)


def kernel(x1, x2):
    _install_axon_trace_shim()
    nc = _get_nc()
    in_maps = _prep_inputs(x1, x2)
    res = run_bass_kernel_spmd(nc, in_maps, core_ids=list(range(B)))
    kernel.last_results = res
    out = np.empty((B, D * D, H, W), dtype=np.float32)
    for b in range(B):
        yb = np.asarray(res.results[b]["y"]).astype(np.float32)
        out[b] = _deskew(yb)
    return out


# revision 9
# speedup vs baseline: 3.0145x; 3.0145x over previous
"""Correlation (FlowNet-style, max_displacement=4) on 8 TRN2 NeuronCores.

Full inputs x1, x2: [B=8, C=64, H=192, W=192] fp32. Output: [8, 81, 192, 192] fp32.
out[b, di*9+dj, h, w] = mean_c x1[b,c,h,w] * x2pad[b,c,h+di,w+dj]   (di,dj in [0,9))

Strategy: batch-parallel (1 batch per core). Per core the correlation is a banded
Gram computed on the TensorEngine with narrow tiles: for each 8x4 (h,w) output
tile (M=32 pixels), one bf16 matmul with lhsT = x1 tile [K=64, M=32] and
rhs = padded x2 window [64, 16x12=192] yields all 81 displacement dot products
of each tile pixel inside a skewed band of the 32x192 PSUM block. Narrow tiles
(TW=4) shrink the PSUM volume that must be evicted by DVE/ACT to 192 elem/pixel
(vs 384 for 8x16 tiles); evictions and output DMAs are kept fully contiguous
(the skew is resolved by a host-side strided view, which is free).

The 128-wide PE array is filled via tiling: the h axis is split into two halves
(channels on partitions 0-63 / 64-127 -> row groups), and four w-tiles are
column-tiled into the four 32-partition PSUM column groups, so 8 narrow matmuls
run concurrently on disjoint PE tiles. A [128, 1024] PSUM tile (2 banks) holds
16 matmuls (4 w-tiles x 4 col groups); one DVE/ACT copy evicts it (fp32->bf16)
into a strip-group ybuf. Only the per-th band rectangles (108 of 192 columns)
are DMA'd out, batched over 4 strips (41.5 KB per DMA). The band is deskewed on
the host with a zero-copy strided view; x1 is pre-scaled by 1/64 (exact).
"""

import sys
import types

import numpy as np
import ml_dtypes

import concourse.bacc as bacc
from concourse import mybir
from concourse.tile import TileContext
from concourse.bass_utils import run_bass_kernel_spmd

B, C, H, W = 8, 64, 192, 192
MAXD = 4
D = 2 * MAXD + 1              # 9
HP, WP = H + 2 * MAXD, W + 2 * MAXD  # 200, 200

TH, TW = 8, 4                 # output tile (h, w) -> M = 32
NH, NW = TH + 2 * MAXD, TW + 2 * MAXD  # 16 x 12 window -> N = 192
N = NH * NW                   # 192 matmul free size
M = TH * TW                   # 32 pixels per matmul
NSP = (H // 2) // TH          # 12 strips per partition-half
NT = W // TW                  # 48 w-tiles
HHALF = H // 2                # 96 rows per half
SLAB = HHALF + 2 * MAXD       # 104 padded x2 rows per half
NG = 3                        # strip groups (4 strips each) per half
KL = D * NW                   # 108 band columns per th-group

# Input h-chunking: strip ranges per chunk and the x2 slab rows they need.
X1_CHUNKS = [(0, 3), (3, 9), (9, 12)]
X2_CHUNKS = [(0, 40), (24, 88), (72, 104)]

BF16 = ml_dtypes.bfloat16


def _install_axon_trace_shim():
    """The image's antenv package lacks axon_hooks; run_bass_kernel_spmd
    crashes on import when trace=True. Provide the hook from the boot module
    so tracing works instead of raising."""
    if "antenv.axon_hooks" in sys.modules:
        return
    try:
        import trn_agent_boot.trn_boot as tb

        hook = tb._ntff_profile_via_ctypes("/opt/axon/libaxon_pjrt.so")
    except Exception:
        hook = None
    mod = types.ModuleType("antenv.axon_hooks")
    mod.get_axon_ntff_profile_hook = lambda: hook
    mod.set_axon_ntff_profile_hook = lambda h: None
    sys.modules["antenv.axon_hooks"] = mod


def build_nc():
    nc = bacc.Bacc("TRN2", target_bir_lowering=False, debug=False)
    # x1 arrives pre-tiled: [128 = half*64+c, strip, wtile, 32 pixels] — walrus
    # requires the matmul weights AP to have a single free dimension.
    x1s = nc.dram_tensor("x1s", [128, NSP, NT, M], mybir.dt.bfloat16, kind="ExternalInput")
    x2s = nc.dram_tensor("x2s", [128, SLAB, WP], mybir.dt.bfloat16, kind="ExternalInput")
    y = nc.dram_tensor("y", [NG, 2, 128, 4, 12, N], mybir.dt.bfloat16,
                       kind="ExternalOutput")

    with TileContext(nc) as tc:
        with (
            tc.tile_pool(name="imgs", bufs=1) as imgs,
            tc.tile_pool(name="outs", bufs=2) as outs,
            tc.tile_pool(name="psum", bufs=4, space="PSUM") as psum,
        ):
            # Chunked input tiles (separate tiles -> precise chunk->matmul deps).
            x1c, x2c = [], []
            for ci in range(3):
                s0, s1 = X1_CHUNKS[ci]
                r0, r1 = X2_CHUNKS[ci]
                x2t = imgs.tile([128, r1 - r0, WP], mybir.dt.bfloat16, tag=f"x2c{ci}")
                nc.sync.dma_start(out=x2t[:], in_=x2s[:, r0:r1, :])
                x1t = imgs.tile([128, s1 - s0, NT, M], mybir.dt.bfloat16, tag=f"x1c{ci}")
                nc.sync.dma_start(out=x1t[:], in_=x1s[:, s0:s1])
                x2c.append(x2t)
                x1c.append(x1t)

            copy_k = 0
            for g in range(NG):
                # ybuf: [128, strip-in-group, tau, n] — tau = local w-tile index;
                # global w-tile t = tau*4 + colgroup (colgroup is folded into the
                # partition index, the host deskew accounts for it). n innermost
                # keeps both the eviction and the output DMA fully contiguous.
                ybufs = [outs.tile([128, 4, 12, N], mybir.dt.bfloat16,
                                   name=f"ybuf{half}_{g}", tag=f"ybuf{half}")
                         for half in range(2)]
                for si in range(4):
                    sp = g * 4 + si
                    ci = next(i for i, (s0, s1) in enumerate(X1_CHUNKS) if s0 <= sp < s1)
                    hl = sp * TH - X2_CHUNKS[ci][0]   # row offset within x2 chunk
                    spl = sp - X1_CHUNKS[ci][0]       # strip offset within x1 chunk
                    for jj in range(3):
                        pts = [psum.tile([128, 1024], mybir.dt.float32,
                                         name=f"pt{half}_{sp}_{jj}", tag="pt")
                               for half in range(2)]
                        for aa in range(2):           # PSUM bank within tile
                            for ss in range(2):       # slot within bank
                                tau = 4 * jj + 2 * aa + ss
                                # Interleave halves and col groups: 8 consecutive
                                # matmuls hit 8 disjoint PE (row, col) tile
                                # positions and execute concurrently.
                                for half in range(2):
                                    p0 = 64 * half
                                    off = aa * 512 + ss * N
                                    for c in range(4):
                                        t = tau * 4 + c
                                        nc.tensor.matmul(
                                            pts[half][32 * c:32 * c + 32, off:off + N],
                                            lhsT=x1c[ci][p0:p0 + 64, spl, t, :],
                                            rhs=x2c[ci][p0:p0 + 64, hl:hl + NH,
                                                        t * TW:t * TW + NW],
                                            start=True, stop=True,
                                            tile_position=(p0, 32 * c),
                                        )
                        # Evict both banks with one op; both sides are (nearly)
                        # contiguous so DVE/ACT stream at ~1 elem/cycle. Split
                        # DVE / ACT 4:5 (DVE is slower per element).
                        for half in range(2):
                            src = (pts[half][:]
                                   .rearrange("p (a b) -> p a b", a=2)[:, :, 0:2 * N])
                            dst = (ybufs[half][:, si, 4 * jj:4 * jj + 4, :]
                                   .rearrange("p (a s) k -> p a (s k)", a=2))
                            if copy_k % 9 in (0, 2, 4, 6):
                                nc.vector.tensor_copy(dst, src)
                            else:
                                nc.scalar.copy(dst, src)
                            copy_k += 1
                # Full-tile dump: one big contiguous DMA per (group, half)
                # (2.4 MB, 36864 B per partition). The band subset would be 44%
                # fewer bytes but costs 32x the DMA instructions and descriptor
                # fragmentation; the host deskew slices the band for free.
                for half in range(2):
                    nc.sync.dma_start(out=y[g, half], in_=ybufs[half][:])

    nc.compile()
    return nc


_NC_CACHE = None


def _get_nc():
    global _NC_CACHE
    if _NC_CACHE is None:
        _NC_CACHE = build_nc()
    return _NC_CACHE


def _prep_inputs(x1, x2):
    """Host-side shard prep: scale, pad, split h into partition halves, bf16."""
    in_maps = []
    x1 = np.asarray(x1, dtype=np.float32)
    x2 = np.asarray(x2, dtype=np.float32)
    x1h = (x1 * (1.0 / C)).astype(BF16)
    x2h = x2.astype(BF16)
    for b in range(B):
        # x1: [64, 192, 192] -> pre-tiled [128 = half*64+c, sp, t, th*4+tw]
        a = x1h[b].reshape(C, 2, NSP, TH, NT, TW)
        a = a.transpose(1, 0, 2, 4, 3, 5).reshape(128, NSP, NT, M)
        # x2: pad to [64, 200, 200], two overlapping 104-row slabs
        p = np.zeros((C, HP, WP), dtype=BF16)
        p[:, MAXD:MAXD + H, MAXD:MAXD + W] = x2h[b]
        s = np.stack([p[:, 0:SLAB, :], p[:, HHALF:HHALF + SLAB, :]], axis=0)
        s = s.reshape(2 * C, SLAB, WP)
        in_maps.append({"x1s": np.ascontiguousarray(a), "x2s": np.ascontiguousarray(s)})
    return in_maps


def _deskew(yb):
    """yb: [3, 2, 128, 4, 12, 192] fp32 (one batch) -> [81, 192, 192]."""
    yr = yb.reshape(NG, 2, 4, TH, TW, 4, 12, N)  # g, half, c, th, tw, s, tau, n
    st = yr.strides
    stn = st[7]
    v = np.lib.stride_tricks.as_strided(
        yr,
        shape=(D, D, 2, NG, 4, TH, 12, 4, TW),
        strides=(NW * stn, stn, st[1], st[0], st[5], st[3] + NW * stn,
                 st[6], st[2], st[4] + stn),
    )
    return np.ascontiguousarray(v).reshape(D * D, H, W)


def kernel(x1, x2):
    _install_axon_trace_shim()
    nc = _get_nc()
    in_maps = _prep_inputs(x1, x2)
    res = run_bass_kernel_spmd(nc, in_maps, core_ids=list(range(B)))
    kernel.last_results = res
    out = np.empty((B, D * D, H, W), dtype=np.float32)
    for b in range(B):
        yb = np.asarray(res.results[b]["y"]).astype(np.float32)
        out[b] = _deskew(yb)
    return out
